# revision 37
# baseline (speedup 1.0000x reference)
"""Trainium2 Bass kernel for nn_MAB_44057774522768 (Set-Transformer MAB block).

Reference computation (per batch b, with B=8, Sq=Sk=1024, D=512, H=8 heads,
dh=64):
    Qp = Q @ Wq.T + bq                  [Sq, D]
    Kp = K @ Wk.T + bk                  [Sk, D]
    Vp = K @ Wv.T + bv                  [Sk, D]
    scores_h = Qp_h @ Kp_h.T / sqrt(D)  per head  [Sq, Sk]
    A = softmax(scores, axis=-1)
    ctx_h = A_h @ Vp_h
    O1 = Qp + ctx                       (residual on projected Q)
    out = O1 + relu(O1 @ Wo.T + bo)     (FFN residual)

Sharding: pure data-parallel, batch b -> core b (B == 8 == n_cores).

Device-side layout: "feature-major" — activations stored transposed
[feature, seq] so every matmul contracts over the partition axis with zero
on-chip transposes.  All matmul operands are bf16 (PSUM accumulates fp32).

The attend loop is a 3-engine software pipeline built around the fact that
every engine queue is strict FIFO and PSUM is fully allocated (score ring
2 x [128,1024] + ctx accumulators 2 x [65,1024] = 8 banks), which makes
scores(m+1) wait on exp(m) through the 2-deep score ring:

  - head A's exp on ACT (table exp), head B's on DVE (Schraudolph bit trick:
    int16(x*128/ln2 + 16248.6) viewed as bf16; ~2% element error that washes
    out through the softmax) so the two per-head chains advance in parallel;
  - ctx matmuls are emitted one m behind scores so they execute inside the
    exp-wait window instead of on the chain;
  - each pair's normalization tail is handed to the NEXT pair and emitted
    mid-loop (m=2..4) so its ACT/DVE ops never head-of-line block the next
    pair's exps, and all SBUF-only tail math (ctx*recip, +residual) runs on
    GPSIMD, off both chain engines;
  - projections run upfront (interleaving them as attend "fillers" stalls
    the exp queues: their PSUM->SBUF eviction op has to sit between exps).

The softmax denominator rides as a ones-column appended to V (row 64 of the
ctx PSUM accumulator).  bv is NOT added to Vp on-device: softmax weights
sum to 1, so A @ (Vp + bv) == A @ Vp + bv; bv is folded into the FFN bias
(bo2 = bo + Wo @ bv) on the host.
"""

import math
import os

import numpy as np

import concourse.bass as bass
import concourse.mybir as mybir
import concourse.tile as tile
from concourse import bacc
from concourse.bass_utils import run_bass_kernel_spmd

B, SQ, SK, D = 8, 1024, 1024, 512
H, DH = 8, 64
N_CORES = 8
KC = D // 128  # 4 contraction chunks of 128 (din)
MT = D // 128  # 4 output-feature tiles of 128 (dout)
NQ = SQ // 512  # 2 moving chunks of 512 (seq)
KT8 = SK // 128  # 8 key-seq tiles of 128

F32 = mybir.dt.float32
BF16 = mybir.dt.bfloat16
F8E4 = mybir.dt.float8e4
I16 = mybir.dt.int16
I8 = mybir.dt.int8
ALU = mybir.AluOpType
ACTF = mybir.ActivationFunctionType

_NC = None


def _build():
    # This image's default backend options carry --enable-ldw-opt=false,
    # which keeps walrus from eliding redundant LDWEIGHTS.  Every matmul
    # here re-loads its stationary operand (~70-100ns exposed per matmul),
    # so flip it on; loops below order same-stationary matmuls adjacently
    # to give the pass fodder.
    if os.environ.get("KLDWOPT", "1") == "1":
        from concourse.compiler_utils import get_compiler_flags, set_compiler_flags

        flags = [
            f.replace("--enable-ldw-opt=false", "--enable-ldw-opt=true")
            for f in get_compiler_flags()
        ]
        set_compiler_flags(flags)

    nc = bacc.Bacc(None, target_bir_lowering=False, debug=False)

    # K-side inputs ship as fp8e4: Kp only feeds the (exp-compressed) scores
    # and Vp feeds the ctx term, which is ~30x smaller than the Q-residual
    # it adds into, so their ~4% rms quantization noise stays invisible at
    # the output.  Q/Wq/Wo stay bf16 (the Qp residual reaches the output
    # directly).  Shaves ~1.2MB off the startup input-DMA ramp.
    dQT = nc.dram_tensor("QT", [D, SQ], BF16, kind="ExternalInput")
    dKT = nc.dram_tensor("KT", [D, SK], F8E4, kind="ExternalInput")
    dWq = nc.dram_tensor("WqT", [D, D], BF16, kind="ExternalInput")  # [din,dout]
    dWk = nc.dram_tensor("WkT", [D, D], F8E4, kind="ExternalInput")
    dWv = nc.dram_tensor("WvT", [D, D], F8E4, kind="ExternalInput")
    dWo = nc.dram_tensor("WoT", [D, D], BF16, kind="ExternalInput")
    # all four bias vectors in one tensor / one DMA: [128, (bk|bq|bo2|bv)]
    dBIAS = nc.dram_tensor("BIAS", [128, 4, MT], F32, kind="ExternalInput")
    dOT = nc.dram_tensor("OT", [D, SQ], F32, kind="ExternalOutput")

    dbg = os.environ.get("KDEBUG", "0") == "1"
    if dbg:
        dDQP = nc.dram_tensor("DQP", [128, MT, SQ], BF16, kind="ExternalOutput")
        dDKP = nc.dram_tensor("DKP", [128, MT, SK], BF16, kind="ExternalOutput")
        dDVPA = nc.dram_tensor("DVPA", [128, KT8, H, DH + 2], F8E4, kind="ExternalOutput")
        dDEXA = nc.dram_tensor("DEXA", [128, SQ], BF16, kind="ExternalOutput")
        dDEXB = nc.dram_tensor("DEXB", [128, SQ], BF16, kind="ExternalOutput")
        dDRB = nc.dram_tensor("DRB", [128, SQ], F32, kind="ExternalOutput")
        dDCB = nc.dram_tensor("DCB", [128, SQ], F32, kind="ExternalOutput")
        dDCN = nc.dram_tensor("DCN", [128, SQ], BF16, kind="ExternalOutput")
        dDO1 = nc.dram_tensor("DO1", [128, MT, SQ], BF16, kind="ExternalOutput")

    scale = 1.0 / math.sqrt(float(D))
    # Schraudolph exp in fp8e4m3 bit-space (3-bit mantissa, bias 7): the
    # int8 bit pattern of e4m3(exp(x)) is ~ x*8/ln2 + (7*8 - 0.0579*8).
    sch8_mul = (2.0**3 / math.log(2.0)) * scale
    sch8_add = 7.0 * 8.0 - 0.0579 * 8.0

    with tile.TileContext(nc) as tc:
        with (
            tc.tile_pool(name="persist", bufs=1) as persist,
            tc.tile_pool(name="spool", bufs=2, space="PSUM") as spool,
            tc.tile_pool(name="cpool", bufs=2, space="PSUM") as cpool,
            tc.tile_pool(name="epool", bufs=12) as epool,
            tc.tile_pool(name="rpool", bufs=2) as rpool,
            tc.tile_pool(name="pairpool", bufs=2) as pairpool,
            tc.tile_pool(name="outpool", bufs=2) as outpool,
            tc.tile_pool(name="dpool", bufs=2, space="DRAM") as dpool,
        ):
            # ---- persistent SBUF tensors ----
            qt = persist.tile([128, KC, SQ], BF16)
            kt = persist.tile([128, KC, SK], F8E4)
            wq = persist.tile([128, KC, D], BF16)
            wk = persist.tile([128, KC, D], F8E4)
            wv = persist.tile([128, KC, D], F8E4)
            wo = persist.tile([128, KC, D], BF16)
            bias4 = persist.tile([128, 4, MT], F32)
            bk = bias4[:, 0, :]
            bq = bias4[:, 1, :]
            bo2 = bias4[:, 2, :]
            bv = bias4[:, 3, :]
            qpb = persist.tile([128, MT, SQ], BF16)
            kpb = persist.tile([128, MT, SK], BF16)
            # Vp in seq-major [k, h, dh] + ones column at dh=64 per head;
            # fp8 so the ctx matmul can run in DoubleRow perf mode.  Width
            # padded to 66 so the ktile stride (H*66=528) satisfies the dual-
            # fp8 LDWEIGHTS step%16==0 ISA rule; the pad column just lands in
            # psum row 65, which nothing reads.
            vpa = persist.tile([128, KT8, H, DH + 2], F8E4)
            o1 = persist.tile([128, MT, SQ], BF16)

            # ---- input DMAs ----
            # One folded DMA per (tensor, queue-half): dst [128, kc, row] <-
            # src rows kc*128+p.  Four queues balanced by first-need time so
            # the K-projection inputs (fp8, smallest) land first and K-proj
            # matmuls start ~9-10us while the bf16 Q side is still in flight.
            def fold_src(dt_, kc0, kcn, row_elems):
                base = dt_[kc0 * 128:(kc0 + kcn) * 128, :]
                return bass.AP(
                    tensor=base.tensor,
                    offset=base.offset,
                    ap=[[row_elems, 128], [128 * row_elems, kcn], [1, row_elems]],
                )

            nc.sync.dma_start(out=wk, in_=fold_src(dWk, 0, KC, D))
            nc.scalar.dma_start(out=kt[:, 0:2, :], in_=fold_src(dKT, 0, 2, SK))
            nc.gpsimd.dma_start(out=bias4, in_=dBIAS[:, :, :])
            nc.scalar.dma_start(out=kt[:, 2:4, :], in_=fold_src(dKT, 2, 2, SK))
            nc.sync.dma_start(out=wq, in_=fold_src(dWq, 0, KC, D))
            nc.gpsimd.dma_start(out=qt[:, 2:4, :], in_=fold_src(dQT, 2, 2, SQ))
            nc.scalar.dma_start(out=qt[:, 0:2, :], in_=fold_src(dQT, 0, 2, SQ))
            nc.gpsimd.dma_start(out=wv, in_=fold_src(dWv, 0, KC, D))
            nc.sync.dma_start(out=wo, in_=fold_src(dWo, 0, KC, D))

            # ones column for the fused softmax denominator (+ zeroed pad col)
            nc.vector.memset(vpa[:, :, :, DH:DH + 1], 1.0)
            nc.vector.memset(vpa[:, :, :, DH + 1:DH + 2], 0.0)

            _pp_flip = [0]

            def _work_pair(name):
                """Two [128,512] psum half-tiles (1 bank each).  All psum work
                outside the ctx accumulators runs at seq-half granularity so
                the whole kernel fits in 4 half-bank tags + 2 ctx tiles = 8
                banks, which is what lets the attend score rings double-buffer
                (exp of half n overlaps the scores matmul of half n+1)."""
                _pp_flip[0] ^= 1
                side = "b" if _pp_flip[0] else "a"
                return [
                    spool.tile([128, 512], F32, name=f"{name}{side}{n}",
                               tag=f"s{side}{n}", bufs=1)
                    for n in range(NQ)
                ]

            def _evict_eng():
                # alternate the PSUM->SBUF eviction between ACT and DVE so the
                # two 1-deep projection rings drain through independent engines
                return nc.scalar if _pp_flip[0] else nc.vector

            def project(dst, w, rhs_src, bias_ap, m, dr=False):
                """dst[:, m, :] = (w[:,:,m-tile].T @ rhs_src) + bias.

                dr=True (both operands fp8): DoubleRow perf mode contracts two
                128-deep kc subtiles per instruction at 2 rows/cycle."""
                pp = _work_pair("pp")
                if dr:
                    for kcp in range(KC // 2):
                        ksl = slice(2 * kcp, 2 * kcp + 2)
                        for n in range(NQ):
                            nsl = slice(n * 512, (n + 1) * 512)
                            nc.tensor.matmul(
                                pp[n][:, :],
                                w[:, ksl, m * 128:(m + 1) * 128],
                                rhs_src[:, ksl, nsl],
                                start=(kcp == 0),
                                stop=(kcp == KC // 2 - 1),
                                perf_mode=mybir.MatmulPerfMode.DoubleRow,
                            )
                else:
                    for kc in range(KC):
                        for n in range(NQ):
                            nsl = slice(n * 512, (n + 1) * 512)
                            nc.tensor.matmul(
                                pp[n][:, :],
                                w[:, kc, m * 128:(m + 1) * 128],
                                rhs_src[:, kc, nsl],
                                start=(kc == 0),
                                stop=(kc == KC - 1),
                            )
                eng = _evict_eng()
                for n in range(NQ):
                    nsl = slice(n * 512, (n + 1) * 512)
                    if eng is nc.scalar:
                        eng.activation(
                            dst[:, m, nsl], pp[n][:, :], ACTF.Identity, bias=bias_ap
                        )
                    else:
                        eng.tensor_scalar(
                            dst[:, m, nsl], pp[n][:, :], bias_ap, None, ALU.add
                        )

            def project_v2(mtp):
                """vpa[:, 2mtp:2mtp+2, :, 0:64] = Vp for two key tiles."""
                pv = _work_pair("pv")
                for j in range(2):
                    mt = 2 * mtp + j
                    for kcp in range(KC // 2):
                        ksl = slice(2 * kcp, 2 * kcp + 2)
                        nc.tensor.matmul(
                            pv[j][:, :],
                            kt[:, ksl, mt * 128:(mt + 1) * 128],
                            wv[:, ksl, :],
                            start=(kcp == 0),
                            stop=(kcp == KC // 2 - 1),
                            perf_mode=mybir.MatmulPerfMode.DoubleRow,
                        )
                eng = _evict_eng()
                for j in range(2):
                    mt = 2 * mtp + j
                    src = pv[j][:, :].rearrange("p (h d) -> p h d", h=H)
                    if eng is nc.scalar:
                        eng.activation(vpa[:, mt, :, 0:DH], src, ACTF.Copy)
                    else:
                        eng.tensor_copy(vpa[:, mt, :, 0:DH], src)

            def attend_pair(t, tail_prev):
                """Heads 2t (ACT exp) and 2t+1 (mostly-DVE exp).  Head B runs
                one key-tile step BEHIND head A so a late exp on one chain
                doesn't stall the other chain's scores in the PE's in-order
                queue.  Scores run at seq-half granularity into per-half 1-bank
                rings (sa0/sa1, sb0/sb1): the exp of half n overlaps the
                scores matmul of half n+1, so each chain is paced by its exp
                engine's throughput, not the scores->exp->scores round trip.
                A/B score matmuls are also emitted adjacently and land in
                disjoint PE row groups (h0/h64), so the two heads' 64-deep
                matmuls execute concurrently in the array."""
                pca = cpool.tile([128, SQ], F32, name="pca", tag="pc")
                pcb = cpool.tile([128, SQ], F32, name="pcb", tag="pc")

                def emit_ctx_pair(pc, p, epair, h):
                    """ctx += A[ktiles 2p,2p+1] @ V via one fp8 DoubleRow
                    matmul per seq half (2 key tiles contracted at 2/cycle)."""
                    for n in range(NQ):
                        nsl = slice(n * 512, (n + 1) * 512)
                        nc.tensor.matmul(
                            pc[0:DH + 2, nsl],
                            vpa[:, 2 * p:2 * p + 2, h, :],
                            epair[:, :, nsl],
                            start=(p == 0), stop=(p == KT8 // 2 - 1),
                            perf_mode=mybir.MatmulPerfMode.DoubleRow,
                        )

                def score_half(ps, hb, m, n):
                    nc.tensor.matmul(
                        ps[:, :],
                        kpb[hb:hb + 64, t, m * 128:(m + 1) * 128],
                        qpb[hb:hb + 64, t, n * 512:(n + 1) * 512],
                        start=True, stop=True,
                    )

                penda, pendb = [], []
                exa = exb = None
                for step in range(KT8 + 1):
                    ma, mb = step, step - 1
                    if ma < KT8:
                        if ma % 2 == 0:
                            exa = epool.tile([128, 2, SQ], F8E4, name="exa", tag="ex")
                        for n in range(NQ):
                            nsl = slice(n * 512, (n + 1) * 512)
                            psa = spool.tile(
                                [128, 512], F32, name=f"psa{n}", tag=f"sa{n}", bufs=1
                            )
                            score_half(psa, 0, ma, n)
                            nc.scalar.activation(
                                exa[:, ma % 2, nsl], psa[:, :], ACTF.Exp, scale=scale
                            )
                        if ma % 2 == 1:
                            penda.append((ma // 2, exa))
                    if mb >= 0:
                        if mb % 2 == 0:
                            exb = epool.tile([128, 2, SQ], F8E4, name="exb", tag="ex")
                        for n in range(NQ):
                            nsl = slice(n * 512, (n + 1) * 512)
                            psb = spool.tile(
                                [128, 512], F32, name=f"psb{n}", tag=f"sb{n}", bufs=1
                            )
                            score_half(psb, 64, mb, n)
                            if mb in (3, 6):
                                # DVE (exp + recip + normalize) carries more
                                # than ACT; shift two B-exps over to balance
                                nc.scalar.activation(
                                    exb[:, mb % 2, nsl], psb[:, :], ACTF.Exp,
                                    scale=scale,
                                )
                            else:
                                nc.vector.tensor_scalar(
                                    exb.bitcast(I8)[:, mb % 2, nsl], psb[:, :],
                                    sch8_mul, sch8_add, ALU.mult, ALU.add,
                                )
                        if mb % 2 == 1:
                            pendb.append((mb // 2, exb))
                    if tail_prev is not None:
                        if step == 2:
                            tail_prev[0]()
                        elif step == 3:
                            tail_prev[1]()
                        elif step == 4:
                            tail_prev[2]()
                    # drain ctx (ktile-pair granularity) behind scores; with a
                    # handed-off tail, hold the backlog until the pc banks are
                    # freed by the tail's last reads (pcb: s1 evict @2;
                    # pca: s3 psum-direct multiply @4)
                    ok_a = tail_prev is None or step >= 5
                    ok_b = tail_prev is None or step >= 3
                    if ok_a:
                        while len(penda) > (1 if ma < KT8 else 0):
                            p_, ex_ = penda.pop(0)
                            emit_ctx_pair(pca, p_, ex_, 2 * t)
                    if ok_b:
                        while len(pendb) > (1 if mb < KT8 - 1 else 0):
                            p_, ex_ = pendb.pop(0)
                            emit_ctx_pair(pcb, p_, ex_, 2 * t + 1)

                # ---- this pair's tail (emitted by the NEXT pair) ----
                # Head B's ctx+den evicts to SBUF (ACT) because its rows must
                # cross partitions (SBUF->SBUF DMA shift).  Head A's ctx stays
                # in PSUM: the den reciprocals read PSUM directly on DVE (the
                # recip doubles as the evict), 1/den partition-broadcasts via
                # a DRAM bounce, and the normalize multiplies read pca from
                # PSUM (DVE) / cb from SBUF (GPSIMD).
                cb = pairpool.tile([128, SQ], F32, name="cb", tag="cb")
                rb = pairpool.tile([128, SQ], F32, name="rb", tag="rb")
                scra = rpool.tile([128, SQ], F32, name="scra", tag="scra")
                scrb = rpool.tile([128, SQ], F32, name="scrb", tag="scrb")
                cn = pairpool.tile([128, SQ], BF16, name="cn", tag="cn")

                def s1():
                    nc.scalar.activation(cb[0:DH + 1, :], pcb[0:DH + 1, :], ACTF.Copy)

                def s2():
                    # NOTE: the custom-DVE reciprocal op silently corrupts at
                    # a non-zero base partition, so run it over rows 0..64
                    # (same cost: DVE time scales with free size, not rows)
                    # and use only the den row 64 downstream.
                    nc.vector.reciprocal_approx_fast(
                        scra[0:DH + 1, :], pca[0:DH + 1, :]
                    )
                    nc.vector.reciprocal_approx_fast(
                        scrb[0:DH + 1, :], cb[0:DH + 1, :]
                    )
                    for hh, scr in ((0, scra), (1, scrb)):
                        rec_d = dpool.tile([1, SQ], F32, name="rec_d", tag="rec_d")
                        nc.sync.dma_start(out=rec_d[:, :], in_=scr[DH:DH + 1, :])
                        bsrc = bass.AP(
                            tensor=rec_d[0:1, :].tensor,
                            offset=rec_d[0:1, :].offset,
                            ap=[[0, 64], [1, SQ]],
                        )
                        nc.sync.dma_start(out=rb[64 * hh:64 * hh + 64, :], in_=bsrc)
                    nc.gpsimd.dma_start(out=cb[64:128, :], in_=cb[0:64, :])

                def s3():
                    if t == 3:
                        # last pair: normalize in 512-halves so the FFN's
                        # kc3 matmuls start on the first half immediately
                        for n in range(NQ):
                            nsl = slice(n * 512, (n + 1) * 512)
                            nc.vector.tensor_mul(
                                cn[0:64, nsl], pca[0:DH, nsl], rb[0:64, nsl]
                            )
                            nc.gpsimd.tensor_mul(
                                cn[64:128, nsl], cb[64:128, nsl], rb[64:128, nsl]
                            )
                            nc.vector.tensor_add(o1[:, t, nsl], cn[:, nsl], qpb[:, t, nsl])
                    else:
                        nc.vector.tensor_mul(cn[0:64, :], pca[0:DH, :], rb[0:64, :])
                        nc.gpsimd.tensor_mul(
                            cn[64:128, :], cb[64:128, :], rb[64:128, :]
                        )
                        nc.gpsimd.tensor_add(o1[:, t, :], cn[:, :], qpb[:, t, :])
                    if dbg and t == 0:
                        nc.sync.dma_start(out=dDRB[:, :], in_=rb[:, :])
                        nc.sync.dma_start(out=dDCB[:, :], in_=cb[:, :])
                        nc.sync.dma_start(out=dDCN[:, :], in_=cn[:, :])

                return [s1, s2, s3]

            # ---- FFN: out = O1 + bv + relu(WoT.T @ O1 + bo2) ----
            def ffn_p1(m, pf=None):
                """kc0-2 partial passes into a pair of [128,512] psum halves."""
                if pf is None:
                    pf = _work_pair("pf")
                for kc in range(KC - 1):
                    for n in range(NQ):
                        nsl = slice(n * 512, (n + 1) * 512)
                        nc.tensor.matmul(
                            pf[n][:, :],
                            wo[:, kc, m * 128:(m + 1) * 128],
                            o1[:, kc, nsl],
                            start=(kc == 0),
                            stop=False,
                        )
                return pf

            def ffn_p2(m, pf):
                """kc3 pass + relu/residual/store, pipelined in 512-halves so
                the store of the first half overlaps the math of the second."""
                rf = outpool.tile([128, SQ], F32, name="rf", tag="rf")
                ot = outpool.tile([128, SQ], F32, name="ot", tag="ot")
                for n in range(NQ):
                    nsl = slice(n * 512, (n + 1) * 512)
                    nc.tensor.matmul(
                        pf[n][:, :],
                        wo[:, KC - 1, m * 128:(m + 1) * 128],
                        o1[:, KC - 1, nsl],
                        start=False,
                        stop=True,
                    )
                    nc.scalar.activation(
                        rf[:, nsl], pf[n][:, :], ACTF.Relu, bias=bo2[:, m:m + 1]
                    )
                    nc.vector.scalar_tensor_tensor(
                        ot[:, nsl], rf[:, nsl], bv[:, m:m + 1], o1[:, m, nsl],
                        ALU.add, ALU.add,
                    )
                    eng = nc.gpsimd if (m + n) % 2 == 0 else nc.sync
                    eng.dma_start(
                        out=dOT[m * 128:(m + 1) * 128, nsl], in_=ot[:, nsl]
                    )

            # ---- emission ----
            # all of K-proj first (its inputs land first); Q/V after
            project(kpb, wk, kt, bk[:, 0:1], 0, dr=True)
            project(kpb, wk, kt, bk[:, 1:2], 1, dr=True)
            project(kpb, wk, kt, bk[:, 2:3], 2, dr=True)
            project(kpb, wk, kt, bk[:, 3:4], 3, dr=True)
            project(qpb, wq, qt, bq[:, 0:1], 0)
            project_v2(0)
            project(qpb, wq, qt, bq[:, 1:2], 1)
            project_v2(1)
            project(qpb, wq, qt, bq[:, 2:3], 2)
            project_v2(2)
            project(qpb, wq, qt, bq[:, 3:4], 3)
            project_v2(3)

            tail = attend_pair(0, None)
            tail = attend_pair(1, tail)
            tail = attend_pair(2, tail)
            tail = attend_pair(3, tail)
            # interleave the last pair's tail with ALL FFN kc0-2 passes: the
            # tail's recip/broadcast/normalize latency (several us) is hidden
            # behind 24 matmuls that only need o1[:, 0:3, :], and no ffn_p2
            # (which needs o1[:, 3, :]) sits in the PE queue ahead of them.
            tail[0]()
            pf0 = ffn_p1(0)
            tail[1]()
            pf1 = ffn_p1(1)
            tail[2]()
            # m=2/3 partials reuse the last attend pair's (now drained)
            # ctx-accumulator banks; m=0/1 hold the four score-ring half-banks
            _pfull2 = cpool.tile([128, SQ], F32, name="pf2", tag="pc")
            pf2 = ffn_p1(2, [_pfull2[:, 0:512], _pfull2[:, 512:1024]])
            _pfull3 = cpool.tile([128, SQ], F32, name="pf3", tag="pc")
            pf3 = ffn_p1(3, [_pfull3[:, 0:512], _pfull3[:, 512:1024]])

            if dbg:
                nc.sync.dma_start(out=dDQP[:, :, :], in_=qpb[:, :, :])
                nc.sync.dma_start(out=dDKP[:, :, :], in_=kpb[:, :, :])
                nc.sync.dma_start(out=dDVPA[:, :, :, :], in_=vpa[:, :, :, :])
                nc.sync.dma_start(out=dDO1[:, :, :], in_=o1[:, :, :])

            ffn_p2(0, pf0)
            ffn_p2(1, pf1)
            ffn_p2(2, pf2)
            ffn_p2(3, pf3)

    nc.compile()
    return nc


def _get_nc():
    global _NC
    if _NC is None:
        _NC = _build()
    return _NC


def _prep_inputs(Q, K, Wq, bq, Wk, bk, Wv, bv, Wo, bo):
    Q = np.asarray(Q, dtype=np.float32)
    K = np.asarray(K, dtype=np.float32)
    Wq = np.asarray(Wq, dtype=np.float32)
    Wk = np.asarray(Wk, dtype=np.float32)
    Wv = np.asarray(Wv, dtype=np.float32)
    Wo = np.asarray(Wo, dtype=np.float32)
    bq = np.asarray(bq, dtype=np.float32)
    bk = np.asarray(bk, dtype=np.float32)
    bv = np.asarray(bv, dtype=np.float32)
    bo = np.asarray(bo, dtype=np.float32)

    bo2 = (bo + Wo @ bv).astype(np.float32)

    def btile(b):
        return np.ascontiguousarray(b.reshape(MT, 128).T)

    import ml_dtypes
    bf = ml_dtypes.bfloat16
    f8 = mybir.dt.np(F8E4)
    shared = {
        "WqT": np.ascontiguousarray(Wq.T).astype(bf),
        "WkT": np.ascontiguousarray(Wk.T).astype(f8),
        "WvT": np.ascontiguousarray(Wv.T).astype(f8),
        "WoT": np.ascontiguousarray(Wo.T).astype(bf),
        "BIAS": np.ascontiguousarray(
            np.stack([btile(bk), btile(bq), btile(bo2), btile(bv)], axis=1)
        ),
    }
    in_maps = []
    for c in range(N_CORES):
        m = dict(shared)
        m["QT"] = np.ascontiguousarray(Q[c].T).astype(bf)
        m["KT"] = np.ascontiguousarray(K[c].T).astype(f8)
        in_maps.append(m)
    return in_maps


def run(inputs, trace=False):
    """Run on hardware; returns (output [B,SQ,D] f32, BassKernelResults)."""
    in_maps = _prep_inputs(
        inputs["Q"], inputs["K"], inputs["Wq"], inputs["bq"], inputs["Wk"],
        inputs["bk"], inputs["Wv"], inputs["bv"], inputs["Wo"], inputs["bo"],
    )
    nc = _get_nc()
    res = run_bass_kernel_spmd(
        nc, in_maps, core_ids=list(range(N_CORES)), trace=trace
    )
    out = np.stack(
        [res.results[c]["OT"].T for c in range(N_CORES)], axis=0
    ).astype(np.float32)
    return out, res


def kernel(**inputs):
    nh = inputs.get("num_heads", H)
    assert int(nh) == H, f"kernel hardcodes num_heads={H}, got {nh}"
    out, _ = run(inputs, trace=False)
    return out


if __name__ == "__main__":
    rng = np.random.default_rng(0)
    inputs = {
        "Q": rng.standard_normal((B, SQ, D), dtype=np.float32),
        "K": rng.standard_normal((B, SK, D), dtype=np.float32),
        "Wq": rng.standard_normal((D, D), dtype=np.float32) * 0.04,
        "bq": rng.standard_normal((D,), dtype=np.float32) * 0.04,
        "Wk": rng.standard_normal((D, D), dtype=np.float32) * 0.04,
        "bk": rng.standard_normal((D,), dtype=np.float32) * 0.04,
        "Wv": rng.standard_normal((D, D), dtype=np.float32) * 0.04,
        "bv": rng.standard_normal((D,), dtype=np.float32) * 0.04,
        "Wo": rng.standard_normal((D, D), dtype=np.float32) * 0.04,
        "bo": rng.standard_normal((D,), dtype=np.float32) * 0.04,
        "num_heads": H,
    }
    out = kernel(**inputs)
    print("out", out.shape, out.dtype, float(np.abs(out).max()))



# revision 45
# speedup vs baseline: 1.0462x; 1.0462x over previous
"""Trainium2 Bass kernel for nn_MAB_44057774522768 (Set-Transformer MAB block).

Reference computation (per batch b, with B=8, Sq=Sk=1024, D=512, H=8 heads,
dh=64):
    Qp = Q @ Wq.T + bq                  [Sq, D]
    Kp = K @ Wk.T + bk                  [Sk, D]
    Vp = K @ Wv.T + bv                  [Sk, D]
    scores_h = Qp_h @ Kp_h.T / sqrt(D)  per head  [Sq, Sk]
    A = softmax(scores, axis=-1)
    ctx_h = A_h @ Vp_h
    O1 = Qp + ctx                       (residual on projected Q)
    out = O1 + relu(O1 @ Wo.T + bo)     (FFN residual)

Sharding: pure data-parallel, batch b -> core b (B == 8 == n_cores).

Device-side layout: "feature-major" — activations stored transposed
[feature, seq] so every matmul contracts over the partition axis with zero
on-chip transposes.  All matmul operands are bf16 (PSUM accumulates fp32).

The attend loop is a 3-engine software pipeline built around the fact that
every engine queue is strict FIFO and PSUM is fully allocated (score ring
2 x [128,1024] + ctx accumulators 2 x [65,1024] = 8 banks), which makes
scores(m+1) wait on exp(m) through the 2-deep score ring:

  - head A's exp on ACT (table exp), head B's on DVE (Schraudolph bit trick:
    int16(x*128/ln2 + 16248.6) viewed as bf16; ~2% element error that washes
    out through the softmax) so the two per-head chains advance in parallel;
  - ctx matmuls are emitted one m behind scores so they execute inside the
    exp-wait window instead of on the chain;
  - each pair's normalization tail is handed to the NEXT pair and emitted
    mid-loop (m=2..4) so its ACT/DVE ops never head-of-line block the next
    pair's exps, and all SBUF-only tail math (ctx*recip, +residual) runs on
    GPSIMD, off both chain engines;
  - projections run upfront (interleaving them as attend "fillers" stalls
    the exp queues: their PSUM->SBUF eviction op has to sit between exps).

The softmax denominator rides as a ones-column appended to V (row 64 of the
ctx PSUM accumulator).  bv is NOT added to Vp on-device: softmax weights
sum to 1, so A @ (Vp + bv) == A @ Vp + bv; bv is folded into the FFN bias
(bo2 = bo + Wo @ bv) on the host.
"""

import math
import os

import numpy as np

import concourse.bass as bass
import concourse.mybir as mybir
import concourse.tile as tile
from concourse import bacc
from concourse.bass_utils import run_bass_kernel_spmd

B, SQ, SK, D = 8, 1024, 1024, 512
H, DH = 8, 64
N_CORES = 8
KC = D // 128  # 4 contraction chunks of 128 (din)
MT = D // 128  # 4 output-feature tiles of 128 (dout)
NQ = SQ // 512  # 2 moving chunks of 512 (seq)
KT8 = SK // 128  # 8 key-seq tiles of 128

F32 = mybir.dt.float32
F32R = mybir.dt.float32r
BF16 = mybir.dt.bfloat16
F8E4 = mybir.dt.float8e4
I16 = mybir.dt.int16
I8 = mybir.dt.int8
ALU = mybir.AluOpType
ACTF = mybir.ActivationFunctionType

_NC = None


def _build():
    # This image's default backend options carry --enable-ldw-opt=false,
    # which keeps walrus from eliding redundant LDWEIGHTS.  Every matmul
    # here re-loads its stationary operand (~70-100ns exposed per matmul),
    # so flip it on; loops below order same-stationary matmuls adjacently
    # to give the pass fodder.
    if os.environ.get("KLDWOPT", "1") == "1":
        from concourse.compiler_utils import get_compiler_flags, set_compiler_flags

        flags = [
            f.replace("--enable-ldw-opt=false", "--enable-ldw-opt=true")
            for f in get_compiler_flags()
        ]
        set_compiler_flags(flags)

    nc = bacc.Bacc(None, target_bir_lowering=False, debug=False)

    # K-side inputs ship as fp8e4: Kp only feeds the (exp-compressed) scores
    # and Vp feeds the ctx term, which is ~30x smaller than the Q-residual
    # it adds into, so their ~4% rms quantization noise stays invisible at
    # the output.  Q/Wq/Wo stay bf16 (the Qp residual reaches the output
    # directly).  Shaves ~1.2MB off the startup input-DMA ramp.
    dQT = nc.dram_tensor("QT", [D, SQ], BF16, kind="ExternalInput")
    dKT = nc.dram_tensor("KT", [D, SK], F8E4, kind="ExternalInput")
    dWq = nc.dram_tensor("WqT", [D, D], BF16, kind="ExternalInput")  # [din,dout]
    dWk = nc.dram_tensor("WkT", [D, D], F8E4, kind="ExternalInput")
    dWv = nc.dram_tensor("WvT", [D, D], F8E4, kind="ExternalInput")
    dWo = nc.dram_tensor("WoT", [D, D], BF16, kind="ExternalInput")
    # all four bias vectors in one tensor / one DMA: [128, (bk|bq|bo2|bv)]
    dBIAS = nc.dram_tensor("BIAS", [128, 4, MT], F32, kind="ExternalInput")
    # bf16 output (upcast on host): halves the output-store drain at the end
    # of the kernel; adds <=0.4% rounding, well inside the 2e-2 gate
    dOT = nc.dram_tensor("OT", [D, SQ], BF16, kind="ExternalOutput")

    dbg = os.environ.get("KDEBUG", "0") == "1"
    if dbg:
        dDQP = nc.dram_tensor("DQP", [128, MT, SQ], BF16, kind="ExternalOutput")
        dDKP = nc.dram_tensor("DKP", [128, MT, SK], BF16, kind="ExternalOutput")
        dDVPA = nc.dram_tensor("DVPA", [128, KT8, H, DH + 2], F8E4, kind="ExternalOutput")
        dDEXA = nc.dram_tensor("DEXA", [128, SQ], BF16, kind="ExternalOutput")
        dDEXB = nc.dram_tensor("DEXB", [128, SQ], BF16, kind="ExternalOutput")
        dDRB = nc.dram_tensor("DRB", [128, SQ], F32, kind="ExternalOutput")
        dDCB = nc.dram_tensor("DCB", [128, SQ], F32, kind="ExternalOutput")
        dDCN = nc.dram_tensor("DCN", [128, SQ], BF16, kind="ExternalOutput")
        dDO1 = nc.dram_tensor("DO1", [128, MT, SQ], BF16, kind="ExternalOutput")

    scale = 1.0 / math.sqrt(float(D))
    # Schraudolph exp in fp8e4m3 bit-space (3-bit mantissa, bias 7): the
    # int8 bit pattern of e4m3(exp(x)) is ~ x*8/ln2 + (7*8 - 0.0579*8).
    sch8_mul = (2.0**3 / math.log(2.0)) * scale
    sch8_add = 7.0 * 8.0 - 0.0579 * 8.0

    with tile.TileContext(nc) as tc:
        with (
            tc.tile_pool(name="persist", bufs=1) as persist,
            tc.tile_pool(name="spool", bufs=2, space="PSUM") as spool,
            tc.tile_pool(name="cpool", bufs=2, space="PSUM") as cpool,
            tc.tile_pool(name="epool", bufs=12) as epool,
            tc.tile_pool(name="rpool", bufs=2) as rpool,
            tc.tile_pool(name="pairpool", bufs=2) as pairpool,
            tc.tile_pool(name="outpool", bufs=2) as outpool,
            tc.tile_pool(name="dpool", bufs=2, space="DRAM") as dpool,
        ):
            # ---- persistent SBUF tensors ----
            qt = persist.tile([128, KC, SQ], BF16)
            kt = persist.tile([128, KC, SK], F8E4)
            wq = persist.tile([128, KC, D], BF16)
            wk = persist.tile([128, KC, D], F8E4)
            wv = persist.tile([128, KC, D], F8E4)
            wo = persist.tile([128, KC, D], BF16)
            bias4 = persist.tile([128, 4, MT], F32)
            bk = bias4[:, 0, :]
            bq = bias4[:, 1, :]
            bo2 = bias4[:, 2, :]
            bv = bias4[:, 3, :]
            qpb = persist.tile([128, MT, SQ], BF16)
            kpb = persist.tile([128, MT, SK], BF16)
            # Vp in seq-major [k, h, dh] + ones column at dh=64 per head;
            # fp8 so the ctx matmul can run in DoubleRow perf mode.  Width
            # padded to 66 so the ktile stride (H*66=528) satisfies the dual-
            # fp8 LDWEIGHTS step%16==0 ISA rule; the pad column just lands in
            # psum row 65, which nothing reads.
            vpa = persist.tile([128, KT8, H, DH + 2], F8E4)
            o1 = persist.tile([128, MT, SQ], BF16)

            # ---- input DMAs ----
            # One folded DMA per (tensor, queue-half): dst [128, kc, row] <-
            # src rows kc*128+p.  Four queues balanced by first-need time so
            # the K-projection inputs (fp8, smallest) land first and K-proj
            # matmuls start ~9-10us while the bf16 Q side is still in flight.
            def fold_src(dt_, kc0, kcn, row_elems):
                base = dt_[kc0 * 128:(kc0 + kcn) * 128, :]
                return bass.AP(
                    tensor=base.tensor,
                    offset=base.offset,
                    ap=[[row_elems, 128], [128 * row_elems, kcn], [1, row_elems]],
                )

            nc.sync.dma_start(out=wk, in_=fold_src(dWk, 0, KC, D))
            nc.scalar.dma_start(out=kt[:, 0:2, :], in_=fold_src(dKT, 0, 2, SK))
            nc.gpsimd.dma_start(out=bias4, in_=dBIAS[:, :, :])
            nc.scalar.dma_start(out=kt[:, 2:4, :], in_=fold_src(dKT, 2, 2, SK))
            nc.sync.dma_start(out=wq, in_=fold_src(dWq, 0, KC, D))
            nc.gpsimd.dma_start(out=qt[:, 2:4, :], in_=fold_src(dQT, 2, 2, SQ))
            nc.scalar.dma_start(out=qt[:, 0:2, :], in_=fold_src(dQT, 0, 2, SQ))
            nc.gpsimd.dma_start(out=wv, in_=fold_src(dWv, 0, KC, D))
            nc.sync.dma_start(out=wo, in_=fold_src(dWo, 0, KC, D))

            # ones column for the fused softmax denominator (+ zeroed pad col)
            nc.vector.memset(vpa[:, :, :, DH:DH + 1], 1.0)
            nc.vector.memset(vpa[:, :, :, DH + 1:DH + 2], 0.0)
            # ones row at partition 64 for the last pair's 1/den broadcast
            ones64 = persist.tile([128, 64], F32)
            nc.vector.memset(ones64[DH:DH + 1, :], 1.0)

            _pp_flip = [0]

            def _work_pair(name):
                """Two [128,512] psum half-tiles (1 bank each).  All psum work
                outside the ctx accumulators runs at seq-half granularity so
                the whole kernel fits in 4 half-bank tags + 2 ctx tiles = 8
                banks, which is what lets the attend score rings double-buffer
                (exp of half n overlaps the scores matmul of half n+1)."""
                _pp_flip[0] ^= 1
                side = "b" if _pp_flip[0] else "a"
                return [
                    spool.tile([128, 512], F32, name=f"{name}{side}{n}",
                               tag=f"s{side}{n}", bufs=1)
                    for n in range(NQ)
                ]

            def _evict_eng():
                # alternate the PSUM->SBUF eviction between ACT and DVE so the
                # two 1-deep projection rings drain through independent engines
                return nc.scalar if _pp_flip[0] else nc.vector

            def project(dst, w, rhs_src, bias_ap, m, dr=False):
                """dst[:, m, :] = (w[:,:,m-tile].T @ rhs_src) + bias.

                dr=True (both operands fp8): DoubleRow perf mode contracts two
                128-deep kc subtiles per instruction at 2 rows/cycle."""
                pp = _work_pair("pp")
                if dr:
                    for kcp in range(KC // 2):
                        ksl = slice(2 * kcp, 2 * kcp + 2)
                        for n in range(NQ):
                            nsl = slice(n * 512, (n + 1) * 512)
                            nc.tensor.matmul(
                                pp[n][:, :],
                                w[:, ksl, m * 128:(m + 1) * 128],
                                rhs_src[:, ksl, nsl],
                                start=(kcp == 0),
                                stop=(kcp == KC // 2 - 1),
                                perf_mode=mybir.MatmulPerfMode.DoubleRow,
                            )
                else:
                    for kc in range(KC):
                        for n in range(NQ):
                            nsl = slice(n * 512, (n + 1) * 512)
                            nc.tensor.matmul(
                                pp[n][:, :],
                                w[:, kc, m * 128:(m + 1) * 128],
                                rhs_src[:, kc, nsl],
                                start=(kc == 0),
                                stop=(kc == KC - 1),
                            )
                eng = _evict_eng()
                for n in range(NQ):
                    nsl = slice(n * 512, (n + 1) * 512)
                    if eng is nc.scalar:
                        eng.activation(
                            dst[:, m, nsl], pp[n][:, :], ACTF.Identity, bias=bias_ap
                        )
                    else:
                        eng.tensor_scalar(
                            dst[:, m, nsl], pp[n][:, :], bias_ap, None, ALU.add
                        )

            def project_v2(mtp):
                """vpa[:, 2mtp:2mtp+2, :, 0:64] = Vp for two key tiles."""
                pv = _work_pair("pv")
                for j in range(2):
                    mt = 2 * mtp + j
                    for kcp in range(KC // 2):
                        ksl = slice(2 * kcp, 2 * kcp + 2)
                        nc.tensor.matmul(
                            pv[j][:, :],
                            kt[:, ksl, mt * 128:(mt + 1) * 128],
                            wv[:, ksl, :],
                            start=(kcp == 0),
                            stop=(kcp == KC // 2 - 1),
                            perf_mode=mybir.MatmulPerfMode.DoubleRow,
                        )
                eng = _evict_eng()
                for j in range(2):
                    mt = 2 * mtp + j
                    src = pv[j][:, :].rearrange("p (h d) -> p h d", h=H)
                    if eng is nc.scalar:
                        eng.activation(vpa[:, mt, :, 0:DH], src, ACTF.Copy)
                    else:
                        eng.tensor_copy(vpa[:, mt, :, 0:DH], src)

            def attend_pair(t, tail_prev):
                """Heads 2t (ACT exp) and 2t+1 (mostly-DVE exp).  Head B runs
                one key-tile step BEHIND head A so a late exp on one chain
                doesn't stall the other chain's scores in the PE's in-order
                queue.  Scores run at seq-half granularity into per-half 1-bank
                rings (sa0/sa1, sb0/sb1): the exp of half n overlaps the
                scores matmul of half n+1, so each chain is paced by its exp
                engine's throughput, not the scores->exp->scores round trip.
                A/B score matmuls are also emitted adjacently and land in
                disjoint PE row groups (h0/h64), so the two heads' 64-deep
                matmuls execute concurrently in the array."""
                pca = cpool.tile([128, SQ], F32, name="pca", tag="pc")
                pcb = cpool.tile([128, SQ], F32, name="pcb", tag="pc")

                def emit_ctx_pair(pc, p, epair, h):
                    """ctx += A[ktiles 2p,2p+1] @ V via one fp8 DoubleRow
                    matmul per seq half (2 key tiles contracted at 2/cycle)."""
                    for n in range(NQ):
                        nsl = slice(n * 512, (n + 1) * 512)
                        nc.tensor.matmul(
                            pc[0:DH + 2, nsl],
                            vpa[:, 2 * p:2 * p + 2, h, :],
                            epair[:, :, nsl],
                            start=(p == 0), stop=(p == KT8 // 2 - 1),
                            perf_mode=mybir.MatmulPerfMode.DoubleRow,
                        )

                def score_half(ps, hb, m, n):
                    nc.tensor.matmul(
                        ps[:, :],
                        kpb[hb:hb + 64, t, m * 128:(m + 1) * 128],
                        qpb[hb:hb + 64, t, n * 512:(n + 1) * 512],
                        start=True, stop=True,
                    )

                penda, pendb = [], []
                exa = exb = None
                for step in range(KT8 + 1):
                    ma, mb = step, step - 1
                    if ma < KT8:
                        if ma % 2 == 0:
                            exa = epool.tile([128, 2, SQ], F8E4, name="exa", tag="ex")
                        for n in range(NQ):
                            nsl = slice(n * 512, (n + 1) * 512)
                            psa = spool.tile(
                                [128, 512], F32, name=f"psa{n}", tag=f"sa{n}", bufs=1
                            )
                            score_half(psa, 0, ma, n)
                            nc.scalar.activation(
                                exa[:, ma % 2, nsl], psa[:, :], ACTF.Exp, scale=scale
                            )
                        if ma % 2 == 1:
                            penda.append((ma // 2, exa))
                    if mb >= 0:
                        if mb % 2 == 0:
                            exb = epool.tile([128, 2, SQ], F8E4, name="exb", tag="ex")
                        for n in range(NQ):
                            nsl = slice(n * 512, (n + 1) * 512)
                            psb = spool.tile(
                                [128, 512], F32, name=f"psb{n}", tag=f"sb{n}", bufs=1
                            )
                            score_half(psb, 64, mb, n)
                            if mb in (3, 6):
                                # DVE (exp + recip + normalize) carries more
                                # than ACT; shift two B-exps over to balance
                                nc.scalar.activation(
                                    exb[:, mb % 2, nsl], psb[:, :], ACTF.Exp,
                                    scale=scale,
                                )
                            else:
                                nc.vector.tensor_scalar(
                                    exb.bitcast(I8)[:, mb % 2, nsl], psb[:, :],
                                    sch8_mul, sch8_add, ALU.mult, ALU.add,
                                )
                        if mb % 2 == 1:
                            pendb.append((mb // 2, exb))
                    if tail_prev is not None:
                        if step == 2:
                            tail_prev[0]()
                        elif step == 3:
                            tail_prev[1]()
                        elif step == 4:
                            tail_prev[2]()
                    # drain ctx (ktile-pair granularity) behind scores; with a
                    # handed-off tail, hold the backlog until the pc banks are
                    # freed by the tail's last reads (pcb: s1 evict @2;
                    # pca: s3 psum-direct multiply @4)
                    ok_a = tail_prev is None or step >= 5
                    ok_b = tail_prev is None or step >= 3
                    if ok_a:
                        while len(penda) > (1 if ma < KT8 else 0):
                            p_, ex_ = penda.pop(0)
                            emit_ctx_pair(pca, p_, ex_, 2 * t)
                    if ok_b:
                        while len(pendb) > (1 if mb < KT8 - 1 else 0):
                            p_, ex_ = pendb.pop(0)
                            emit_ctx_pair(pcb, p_, ex_, 2 * t + 1)

                # ---- this pair's tail (emitted by the NEXT pair) ----
                # Head B's ctx+den evicts to SBUF (ACT) because its rows must
                # cross partitions (SBUF->SBUF DMA shift).  Head A's ctx stays
                # in PSUM: the den reciprocals read PSUM directly on DVE (the
                # recip doubles as the evict), 1/den partition-broadcasts via
                # a DRAM bounce, and the normalize multiplies read pca from
                # PSUM (DVE) / cb from SBUF (GPSIMD).
                cb = pairpool.tile([128, SQ], F32, name="cb", tag="cb")
                rb = pairpool.tile([128, SQ], F32, name="rb", tag="rb")
                scra = rpool.tile([128, SQ], F32, name="scra", tag="scra")
                scrb = rpool.tile([128, SQ], F32, name="scrb", tag="scrb")
                cn = pairpool.tile([128, SQ], BF16, name="cn", tag="cn")

                def s1():
                    nc.scalar.activation(cb[0:DH + 1, :], pcb[0:DH + 1, :], ACTF.Copy)

                def s2():
                    # NOTE: the custom-DVE reciprocal op silently corrupts at
                    # a non-zero base partition, so run it over rows 0..64
                    # (same cost: DVE time scales with free size, not rows)
                    # and use only the den row 64 downstream.
                    if t == 3:
                        # Last pair: the whole normalize chain is exposed at
                        # the attend->FFN boundary, so skip the ~5us DRAM
                        # bounce: partition-broadcast 1/den with two tiny
                        # f32r ones-matmuls per half (stationary at PE row
                        # group 64, col groups 0/64), evict to rb, and run the
                        # per-half normalize on DVE immediately.
                        for n in range(NQ):
                            nsl = slice(n * 512, (n + 1) * 512)
                            nc.vector.reciprocal_approx_fast(
                                scra[0:DH + 1, nsl], pca[0:DH + 1, nsl]
                            )
                            nc.vector.reciprocal_approx_fast(
                                scrb[0:DH + 1, nsl], cb[0:DH + 1, nsl]
                            )
                        nc.gpsimd.dma_start(out=cb[64:128, :], in_=cb[0:64, :])
                        rbps = _work_pair("rbps")
                        for n in range(NQ):
                            nsl = slice(n * 512, (n + 1) * 512)
                            for cg, scr in ((0, scra), (64, scrb)):
                                nc.tensor.matmul(
                                    rbps[n][cg:cg + 64, :],
                                    ones64[DH:DH + 1, 0:64],
                                    scr[DH:DH + 1, nsl],
                                    start=True, stop=True,
                                )
                            nc.scalar.activation(
                                rb[:, nsl], rbps[n][:, :], ACTF.Copy
                            )
                            nc.vector.tensor_mul(
                                cn[0:64, nsl], pca[0:DH, nsl], rb[0:64, nsl]
                            )
                            nc.vector.tensor_mul(
                                cn[64:128, nsl], cb[64:128, nsl], rb[64:128, nsl]
                            )
                            nc.vector.tensor_add(
                                o1[:, t, nsl], cn[:, nsl], qpb[:, t, nsl]
                            )
                        return
                    nc.vector.reciprocal_approx_fast(
                        scra[0:DH + 1, :], pca[0:DH + 1, :]
                    )
                    nc.vector.reciprocal_approx_fast(
                        scrb[0:DH + 1, :], cb[0:DH + 1, :]
                    )
                    for hh, scr in ((0, scra), (1, scrb)):
                        rec_d = dpool.tile([1, SQ], F32, name="rec_d", tag="rec_d")
                        nc.sync.dma_start(out=rec_d[:, :], in_=scr[DH:DH + 1, :])
                        bsrc = bass.AP(
                            tensor=rec_d[0:1, :].tensor,
                            offset=rec_d[0:1, :].offset,
                            ap=[[0, 64], [1, SQ]],
                        )
                        nc.sync.dma_start(out=rb[64 * hh:64 * hh + 64, :], in_=bsrc)
                    nc.gpsimd.dma_start(out=cb[64:128, :], in_=cb[0:64, :])

                def s3():
                    if t == 3:
                        return  # folded into s2 for the last pair
                    nc.vector.tensor_mul(cn[0:64, :], pca[0:DH, :], rb[0:64, :])
                    nc.gpsimd.tensor_mul(
                        cn[64:128, :], cb[64:128, :], rb[64:128, :]
                    )
                    nc.gpsimd.tensor_add(o1[:, t, :], cn[:, :], qpb[:, t, :])
                    if dbg and t == 0:
                        nc.sync.dma_start(out=dDRB[:, :], in_=rb[:, :])
                        nc.sync.dma_start(out=dDCB[:, :], in_=cb[:, :])
                        nc.sync.dma_start(out=dDCN[:, :], in_=cn[:, :])

                return [s1, s2, s3]

            # ---- FFN: out = O1 + bv + relu(WoT.T @ O1 + bo2) ----
            def ffn_p1(m, pf=None):
                """kc0-2 partial passes into a pair of [128,512] psum halves."""
                if pf is None:
                    pf = _work_pair("pf")
                for kc in range(KC - 1):
                    for n in range(NQ):
                        nsl = slice(n * 512, (n + 1) * 512)
                        nc.tensor.matmul(
                            pf[n][:, :],
                            wo[:, kc, m * 128:(m + 1) * 128],
                            o1[:, kc, nsl],
                            start=(kc == 0),
                            stop=False,
                        )
                return pf

            def ffn_p2(m, pf):
                """kc3 pass + relu/residual/store, pipelined in 512-halves so
                the store of the first half overlaps the math of the second."""
                rf = outpool.tile([128, SQ], F32, name="rf", tag="rf")
                ot = outpool.tile([128, SQ], BF16, name="ot", tag="ot")
                for n in range(NQ):
                    nsl = slice(n * 512, (n + 1) * 512)
                    nc.tensor.matmul(
                        pf[n][:, :],
                        wo[:, KC - 1, m * 128:(m + 1) * 128],
                        o1[:, KC - 1, nsl],
                        start=False,
                        stop=True,
                    )
                    nc.scalar.activation(
                        rf[:, nsl], pf[n][:, :], ACTF.Relu, bias=bo2[:, m:m + 1]
                    )
                    nc.vector.scalar_tensor_tensor(
                        ot[:, nsl], rf[:, nsl], bv[:, m:m + 1], o1[:, m, nsl],
                        ALU.add, ALU.add,
                    )
                    eng = nc.gpsimd if (m + n) % 2 == 0 else nc.sync
                    eng.dma_start(
                        out=dOT[m * 128:(m + 1) * 128, nsl], in_=ot[:, nsl]
                    )

            # ---- emission ----
            # all of K-proj first (its inputs land first); Q/V after
            project(kpb, wk, kt, bk[:, 0:1], 0, dr=True)
            project(kpb, wk, kt, bk[:, 1:2], 1, dr=True)
            project(kpb, wk, kt, bk[:, 2:3], 2, dr=True)
            project(kpb, wk, kt, bk[:, 3:4], 3, dr=True)
            project(qpb, wq, qt, bq[:, 0:1], 0)
            project_v2(0)
            project(qpb, wq, qt, bq[:, 1:2], 1)
            project_v2(1)
            project(qpb, wq, qt, bq[:, 2:3], 2)
            project_v2(2)
            project(qpb, wq, qt, bq[:, 3:4], 3)
            project_v2(3)

            tail = attend_pair(0, None)
            tail = attend_pair(1, tail)
            tail = attend_pair(2, tail)
            tail = attend_pair(3, tail)
            # interleave the last pair's tail with ALL FFN kc0-2 passes: the
            # tail's recip/broadcast/normalize latency (several us) is hidden
            # behind 24 matmuls that only need o1[:, 0:3, :], and no ffn_p2
            # (which needs o1[:, 3, :]) sits in the PE queue ahead of them.
            tail[0]()
            pf0 = ffn_p1(0)
            tail[1]()
            # tail[1] allocated its rbps broadcast pair on the other ring
            # side; skip that side for pf1 (its banks free after the rb
            # evicts, which are already emitted) instead of colliding with
            # pf0's still-unread side
            _pp_flip[0] ^= 1
            pf1 = ffn_p1(1)
            tail[2]()
            # m=2/3 partials reuse the last attend pair's (now drained)
            # ctx-accumulator banks; m=0/1 hold the four score-ring half-banks
            _pfull2 = cpool.tile([128, SQ], F32, name="pf2", tag="pc")
            pf2 = ffn_p1(2, [_pfull2[:, 0:512], _pfull2[:, 512:1024]])
            _pfull3 = cpool.tile([128, SQ], F32, name="pf3", tag="pc")
            pf3 = ffn_p1(3, [_pfull3[:, 0:512], _pfull3[:, 512:1024]])

            if dbg:
                nc.sync.dma_start(out=dDQP[:, :, :], in_=qpb[:, :, :])
                nc.sync.dma_start(out=dDKP[:, :, :], in_=kpb[:, :, :])
                nc.sync.dma_start(out=dDVPA[:, :, :, :], in_=vpa[:, :, :, :])
                nc.sync.dma_start(out=dDO1[:, :, :], in_=o1[:, :, :])

            ffn_p2(0, pf0)
            ffn_p2(1, pf1)
            ffn_p2(2, pf2)
            ffn_p2(3, pf3)

    nc.compile()
    return nc


def _get_nc():
    global _NC
    if _NC is None:
        _NC = _build()
    return _NC


def _prep_inputs(Q, K, Wq, bq, Wk, bk, Wv, bv, Wo, bo):
    Q = np.asarray(Q, dtype=np.float32)
    K = np.asarray(K, dtype=np.float32)
    Wq = np.asarray(Wq, dtype=np.float32)
    Wk = np.asarray(Wk, dtype=np.float32)
    Wv = np.asarray(Wv, dtype=np.float32)
    Wo = np.asarray(Wo, dtype=np.float32)
    bq = np.asarray(bq, dtype=np.float32)
    bk = np.asarray(bk, dtype=np.float32)
    bv = np.asarray(bv, dtype=np.float32)
    bo = np.asarray(bo, dtype=np.float32)

    bo2 = (bo + Wo @ bv).astype(np.float32)

    def btile(b):
        return np.ascontiguousarray(b.reshape(MT, 128).T)

    import ml_dtypes
    bf = ml_dtypes.bfloat16
    f8 = mybir.dt.np(F8E4)
    shared = {
        "WqT": np.ascontiguousarray(Wq.T).astype(bf),
        "WkT": np.ascontiguousarray(Wk.T).astype(f8),
        "WvT": np.ascontiguousarray(Wv.T).astype(f8),
        "WoT": np.ascontiguousarray(Wo.T).astype(bf),
        "BIAS": np.ascontiguousarray(
            np.stack([btile(bk), btile(bq), btile(bo2), btile(bv)], axis=1)
        ),
    }
    in_maps = []
    for c in range(N_CORES):
        m = dict(shared)
        m["QT"] = np.ascontiguousarray(Q[c].T).astype(bf)
        m["KT"] = np.ascontiguousarray(K[c].T).astype(f8)
        in_maps.append(m)
    return in_maps


def run(inputs, trace=False):
    """Run on hardware; returns (output [B,SQ,D] f32, BassKernelResults)."""
    in_maps = _prep_inputs(
        inputs["Q"], inputs["K"], inputs["Wq"], inputs["bq"], inputs["Wk"],
        inputs["bk"], inputs["Wv"], inputs["bv"], inputs["Wo"], inputs["bo"],
    )
    nc = _get_nc()
    res = run_bass_kernel_spmd(
        nc, in_maps, core_ids=list(range(N_CORES)), trace=trace
    )
    out = np.stack(
        [res.results[c]["OT"].T for c in range(N_CORES)], axis=0
    ).astype(np.float32)
    return out, res


def kernel(**inputs):
    nh = inputs.get("num_heads", H)
    assert int(nh) == H, f"kernel hardcodes num_heads={H}, got {nh}"
    out, _ = run(inputs, trace=False)
    return out


if __name__ == "__main__":
    rng = np.random.default_rng(0)
    inputs = {
        "Q": rng.standard_normal((B, SQ, D), dtype=np.float32),
        "K": rng.standard_normal((B, SK, D), dtype=np.float32),
        "Wq": rng.standard_normal((D, D), dtype=np.float32) * 0.04,
        "bq": rng.standard_normal((D,), dtype=np.float32) * 0.04,
        "Wk": rng.standard_normal((D, D), dtype=np.float32) * 0.04,
        "bk": rng.standard_normal((D,), dtype=np.float32) * 0.04,
        "Wv": rng.standard_normal((D, D), dtype=np.float32) * 0.04,
        "bv": rng.standard_normal((D,), dtype=np.float32) * 0.04,
        "Wo": rng.standard_normal((D, D), dtype=np.float32) * 0.04,
        "bo": rng.standard_normal((D,), dtype=np.float32) * 0.04,
        "num_heads": H,
    }
    out = kernel(**inputs)
    print("out", out.shape, out.dtype, float(np.abs(out).max()))



# revision 46
# speedup vs baseline: 1.0608x; 1.0140x over previous
"""Trainium2 Bass kernel for nn_MAB_44057774522768 (Set-Transformer MAB block).

Reference computation (per batch b, with B=8, Sq=Sk=1024, D=512, H=8 heads,
dh=64):
    Qp = Q @ Wq.T + bq                  [Sq, D]
    Kp = K @ Wk.T + bk                  [Sk, D]
    Vp = K @ Wv.T + bv                  [Sk, D]
    scores_h = Qp_h @ Kp_h.T / sqrt(D)  per head  [Sq, Sk]
    A = softmax(scores, axis=-1)
    ctx_h = A_h @ Vp_h
    O1 = Qp + ctx                       (residual on projected Q)
    out = O1 + relu(O1 @ Wo.T + bo)     (FFN residual)

Sharding: pure data-parallel, batch b -> core b (B == 8 == n_cores).

Device-side layout: "feature-major" — activations stored transposed
[feature, seq] so every matmul contracts over the partition axis with zero
on-chip transposes.  All matmul operands are bf16 (PSUM accumulates fp32).

The attend loop is a 3-engine software pipeline built around the fact that
every engine queue is strict FIFO and PSUM is fully allocated (score ring
2 x [128,1024] + ctx accumulators 2 x [65,1024] = 8 banks), which makes
scores(m+1) wait on exp(m) through the 2-deep score ring:

  - head A's exp on ACT (table exp), head B's on DVE (Schraudolph bit trick:
    int16(x*128/ln2 + 16248.6) viewed as bf16; ~2% element error that washes
    out through the softmax) so the two per-head chains advance in parallel;
  - ctx matmuls are emitted one m behind scores so they execute inside the
    exp-wait window instead of on the chain;
  - each pair's normalization tail is handed to the NEXT pair and emitted
    mid-loop (m=2..4) so its ACT/DVE ops never head-of-line block the next
    pair's exps, and all SBUF-only tail math (ctx*recip, +residual) runs on
    GPSIMD, off both chain engines;
  - projections run upfront (interleaving them as attend "fillers" stalls
    the exp queues: their PSUM->SBUF eviction op has to sit between exps).

The softmax denominator rides as a ones-column appended to V (row 64 of the
ctx PSUM accumulator).  bv is NOT added to Vp on-device: softmax weights
sum to 1, so A @ (Vp + bv) == A @ Vp + bv; bv is folded into the FFN bias
(bo2 = bo + Wo @ bv) on the host.
"""

import math
import os

import numpy as np

import concourse.bass as bass
import concourse.mybir as mybir
import concourse.tile as tile
from concourse import bacc
from concourse.bass_utils import run_bass_kernel_spmd

B, SQ, SK, D = 8, 1024, 1024, 512
H, DH = 8, 64
N_CORES = 8
KC = D // 128  # 4 contraction chunks of 128 (din)
MT = D // 128  # 4 output-feature tiles of 128 (dout)
NQ = SQ // 512  # 2 moving chunks of 512 (seq)
KT8 = SK // 128  # 8 key-seq tiles of 128

F32 = mybir.dt.float32
F32R = mybir.dt.float32r
BF16 = mybir.dt.bfloat16
F8E4 = mybir.dt.float8e4
I16 = mybir.dt.int16
I8 = mybir.dt.int8
ALU = mybir.AluOpType
ACTF = mybir.ActivationFunctionType

_NC = None


def _build():
    # This image's default backend options carry --enable-ldw-opt=false,
    # which keeps walrus from eliding redundant LDWEIGHTS.  Every matmul
    # here re-loads its stationary operand (~70-100ns exposed per matmul),
    # so flip it on; loops below order same-stationary matmuls adjacently
    # to give the pass fodder.
    if os.environ.get("KLDWOPT", "1") == "1":
        from concourse.compiler_utils import get_compiler_flags, set_compiler_flags

        flags = [
            f.replace("--enable-ldw-opt=false", "--enable-ldw-opt=true")
            for f in get_compiler_flags()
        ]
        set_compiler_flags(flags)

    nc = bacc.Bacc(None, target_bir_lowering=False, debug=False)

    # K-side inputs ship as fp8e4: Kp only feeds the (exp-compressed) scores
    # and Vp feeds the ctx term, which is ~30x smaller than the Q-residual
    # it adds into, so their ~4% rms quantization noise stays invisible at
    # the output.  Q/Wq/Wo stay bf16 (the Qp residual reaches the output
    # directly).  Shaves ~1.2MB off the startup input-DMA ramp.
    dQT = nc.dram_tensor("QT", [D, SQ], BF16, kind="ExternalInput")
    dKT = nc.dram_tensor("KT", [D, SK], F8E4, kind="ExternalInput")
    dWq = nc.dram_tensor("WqT", [D, D], BF16, kind="ExternalInput")  # [din,dout]
    dWk = nc.dram_tensor("WkT", [D, D], F8E4, kind="ExternalInput")
    dWv = nc.dram_tensor("WvT", [D, D], F8E4, kind="ExternalInput")
    dWo = nc.dram_tensor("WoT", [D, D], BF16, kind="ExternalInput")
    # all four bias vectors in one tensor / one DMA: [128, (bk|bq|bo2|bv)]
    dBIAS = nc.dram_tensor("BIAS", [128, 4, MT], F32, kind="ExternalInput")
    # bf16 output (upcast on host): halves the output-store drain at the end
    # of the kernel; adds <=0.4% rounding, well inside the 2e-2 gate
    dOT = nc.dram_tensor("OT", [D, SQ], BF16, kind="ExternalOutput")

    dbg = os.environ.get("KDEBUG", "0") == "1"
    if dbg:
        dDQP = nc.dram_tensor("DQP", [128, MT, SQ], BF16, kind="ExternalOutput")
        dDKP = nc.dram_tensor("DKP", [128, MT, SK], BF16, kind="ExternalOutput")
        dDVPA = nc.dram_tensor("DVPA", [128, KT8, H, DH + 2], F8E4, kind="ExternalOutput")
        dDEXA = nc.dram_tensor("DEXA", [128, SQ], BF16, kind="ExternalOutput")
        dDEXB = nc.dram_tensor("DEXB", [128, SQ], BF16, kind="ExternalOutput")
        dDRB = nc.dram_tensor("DRB", [128, SQ], F32, kind="ExternalOutput")
        dDCB = nc.dram_tensor("DCB", [128, SQ], F32, kind="ExternalOutput")
        dDCN = nc.dram_tensor("DCN", [128, SQ], BF16, kind="ExternalOutput")
        dDO1 = nc.dram_tensor("DO1", [128, MT, SQ], BF16, kind="ExternalOutput")

    scale = 1.0 / math.sqrt(float(D))
    # Schraudolph exp in fp8e4m3 bit-space (3-bit mantissa, bias 7): the
    # int8 bit pattern of e4m3(exp(x)) is ~ x*8/ln2 + (7*8 - 0.0579*8).
    sch8_mul = (2.0**3 / math.log(2.0)) * scale
    sch8_add = 7.0 * 8.0 - 0.0579 * 8.0

    with tile.TileContext(nc) as tc:
        with (
            tc.tile_pool(name="persist", bufs=1) as persist,
            tc.tile_pool(name="spool", bufs=2, space="PSUM") as spool,
            tc.tile_pool(name="cpool", bufs=2, space="PSUM") as cpool,
            tc.tile_pool(name="epool", bufs=12) as epool,
            tc.tile_pool(name="rpool", bufs=2) as rpool,
            tc.tile_pool(name="pairpool", bufs=2) as pairpool,
            tc.tile_pool(name="outpool", bufs=2) as outpool,
            tc.tile_pool(name="dpool", bufs=2, space="DRAM") as dpool,
        ):
            # ---- persistent SBUF tensors ----
            qt = persist.tile([128, KC, SQ], BF16)
            kt = persist.tile([128, KC, SK], F8E4)
            wq = persist.tile([128, KC, D], BF16)
            wk = persist.tile([128, KC, D], F8E4)
            wv = persist.tile([128, KC, D], F8E4)
            wo = persist.tile([128, KC, D], BF16)
            bias4 = persist.tile([128, 4, MT], F32)
            bk = bias4[:, 0, :]
            bq = bias4[:, 1, :]
            bo2 = bias4[:, 2, :]
            bv = bias4[:, 3, :]
            qpb = persist.tile([128, MT, SQ], BF16)
            kpb = persist.tile([128, MT, SK], BF16)
            # Vp in seq-major [k, h, dh] + ones column at dh=64 per head;
            # fp8 so the ctx matmul can run in DoubleRow perf mode.  Width
            # padded to 66 so the ktile stride (H*66=528) satisfies the dual-
            # fp8 LDWEIGHTS step%16==0 ISA rule; the pad column just lands in
            # psum row 65, which nothing reads.
            vpa = persist.tile([128, KT8, H, DH + 2], F8E4)
            o1 = persist.tile([128, MT, SQ], BF16)

            # ---- input DMAs ----
            # One folded DMA per (tensor, queue-half): dst [128, kc, row] <-
            # src rows kc*128+p.  Four queues balanced by first-need time so
            # the K-projection inputs (fp8, smallest) land first and K-proj
            # matmuls start ~9-10us while the bf16 Q side is still in flight.
            def fold_src(dt_, kc0, kcn, row_elems):
                base = dt_[kc0 * 128:(kc0 + kcn) * 128, :]
                return bass.AP(
                    tensor=base.tensor,
                    offset=base.offset,
                    ap=[[row_elems, 128], [128 * row_elems, kcn], [1, row_elems]],
                )

            nc.sync.dma_start(out=wk, in_=fold_src(dWk, 0, KC, D))
            nc.scalar.dma_start(out=kt[:, 0:2, :], in_=fold_src(dKT, 0, 2, SK))
            nc.gpsimd.dma_start(out=bias4, in_=dBIAS[:, :, :])
            nc.scalar.dma_start(out=kt[:, 2:4, :], in_=fold_src(dKT, 2, 2, SK))
            nc.sync.dma_start(out=wq, in_=fold_src(dWq, 0, KC, D))
            nc.gpsimd.dma_start(out=qt[:, 2:4, :], in_=fold_src(dQT, 2, 2, SQ))
            nc.scalar.dma_start(out=qt[:, 0:2, :], in_=fold_src(dQT, 0, 2, SQ))
            nc.gpsimd.dma_start(out=wv, in_=fold_src(dWv, 0, KC, D))
            nc.sync.dma_start(out=wo, in_=fold_src(dWo, 0, KC, D))

            # ones column for the fused softmax denominator (+ zeroed pad col)
            nc.vector.memset(vpa[:, :, :, DH:DH + 1], 1.0)
            nc.vector.memset(vpa[:, :, :, DH + 1:DH + 2], 0.0)
            # ones row at partition 64 for the last pair's 1/den broadcast
            ones64 = persist.tile([128, 64], F32)
            nc.vector.memset(ones64[DH:DH + 1, :], 1.0)

            _pp_flip = [0]

            def _work_pair(name):
                """Two [128,512] psum half-tiles (1 bank each).  All psum work
                outside the ctx accumulators runs at seq-half granularity so
                the whole kernel fits in 4 half-bank tags + 2 ctx tiles = 8
                banks, which is what lets the attend score rings double-buffer
                (exp of half n overlaps the scores matmul of half n+1)."""
                _pp_flip[0] ^= 1
                side = "b" if _pp_flip[0] else "a"
                return [
                    spool.tile([128, 512], F32, name=f"{name}{side}{n}",
                               tag=f"s{side}{n}", bufs=1)
                    for n in range(NQ)
                ]

            def _evict_eng():
                # alternate the PSUM->SBUF eviction between ACT and DVE so the
                # two 1-deep projection rings drain through independent engines
                return nc.scalar if _pp_flip[0] else nc.vector

            def project(dst, w, rhs_src, bias_ap, m, dr=False):
                """dst[:, m, :] = (w[:,:,m-tile].T @ rhs_src) + bias.

                dr=True (both operands fp8): DoubleRow perf mode contracts two
                128-deep kc subtiles per instruction at 2 rows/cycle."""
                pp = _work_pair("pp")
                if dr:
                    for kcp in range(KC // 2):
                        ksl = slice(2 * kcp, 2 * kcp + 2)
                        for n in range(NQ):
                            nsl = slice(n * 512, (n + 1) * 512)
                            nc.tensor.matmul(
                                pp[n][:, :],
                                w[:, ksl, m * 128:(m + 1) * 128],
                                rhs_src[:, ksl, nsl],
                                start=(kcp == 0),
                                stop=(kcp == KC // 2 - 1),
                                perf_mode=mybir.MatmulPerfMode.DoubleRow,
                            )
                else:
                    for kc in range(KC):
                        for n in range(NQ):
                            nsl = slice(n * 512, (n + 1) * 512)
                            nc.tensor.matmul(
                                pp[n][:, :],
                                w[:, kc, m * 128:(m + 1) * 128],
                                rhs_src[:, kc, nsl],
                                start=(kc == 0),
                                stop=(kc == KC - 1),
                            )
                eng = _evict_eng()
                for n in range(NQ):
                    nsl = slice(n * 512, (n + 1) * 512)
                    if eng is nc.scalar:
                        eng.activation(
                            dst[:, m, nsl], pp[n][:, :], ACTF.Identity, bias=bias_ap
                        )
                    else:
                        eng.tensor_scalar(
                            dst[:, m, nsl], pp[n][:, :], bias_ap, None, ALU.add
                        )

            def project_v2(mtp):
                """vpa[:, 2mtp:2mtp+2, :, 0:64] = Vp for two key tiles."""
                pv = _work_pair("pv")
                for j in range(2):
                    mt = 2 * mtp + j
                    for kcp in range(KC // 2):
                        ksl = slice(2 * kcp, 2 * kcp + 2)
                        nc.tensor.matmul(
                            pv[j][:, :],
                            kt[:, ksl, mt * 128:(mt + 1) * 128],
                            wv[:, ksl, :],
                            start=(kcp == 0),
                            stop=(kcp == KC // 2 - 1),
                            perf_mode=mybir.MatmulPerfMode.DoubleRow,
                        )
                eng = _evict_eng()
                for j in range(2):
                    mt = 2 * mtp + j
                    src = pv[j][:, :].rearrange("p (h d) -> p h d", h=H)
                    if eng is nc.scalar:
                        eng.activation(vpa[:, mt, :, 0:DH], src, ACTF.Copy)
                    else:
                        eng.tensor_copy(vpa[:, mt, :, 0:DH], src)

            def attend_pair(t, tail_prev):
                """Heads 2t (ACT exp) and 2t+1 (mostly-DVE exp).  Head B runs
                one key-tile step BEHIND head A so a late exp on one chain
                doesn't stall the other chain's scores in the PE's in-order
                queue.  Scores run at seq-half granularity into per-half 1-bank
                rings (sa0/sa1, sb0/sb1): the exp of half n overlaps the
                scores matmul of half n+1, so each chain is paced by its exp
                engine's throughput, not the scores->exp->scores round trip.
                A/B score matmuls are also emitted adjacently and land in
                disjoint PE row groups (h0/h64), so the two heads' 64-deep
                matmuls execute concurrently in the array."""
                pca = cpool.tile([128, SQ], F32, name="pca", tag="pc")
                pcb = cpool.tile([128, SQ], F32, name="pcb", tag="pc")

                def emit_ctx_pair(pc, p, epair, h):
                    """ctx += A[ktiles 2p,2p+1] @ V via one fp8 DoubleRow
                    matmul per seq half (2 key tiles contracted at 2/cycle)."""
                    for n in range(NQ):
                        nsl = slice(n * 512, (n + 1) * 512)
                        nc.tensor.matmul(
                            pc[0:DH + 2, nsl],
                            vpa[:, 2 * p:2 * p + 2, h, :],
                            epair[:, :, nsl],
                            start=(p == 0), stop=(p == KT8 // 2 - 1),
                            perf_mode=mybir.MatmulPerfMode.DoubleRow,
                        )

                def score_half(ps, hb, m, n):
                    nc.tensor.matmul(
                        ps[:, :],
                        kpb[hb:hb + 64, t, m * 128:(m + 1) * 128],
                        qpb[hb:hb + 64, t, n * 512:(n + 1) * 512],
                        start=True, stop=True,
                    )

                penda, pendb = [], []
                exa = exb = None
                for step in range(KT8 + 1):
                    ma, mb = step, step - 1
                    if ma < KT8:
                        if ma % 2 == 0:
                            exa = epool.tile([128, 2, SQ], F8E4, name="exa", tag="ex")
                        for n in range(NQ):
                            nsl = slice(n * 512, (n + 1) * 512)
                            psa = spool.tile(
                                [128, 512], F32, name=f"psa{n}", tag=f"sa{n}", bufs=1
                            )
                            score_half(psa, 0, ma, n)
                            nc.scalar.activation(
                                exa[:, ma % 2, nsl], psa[:, :], ACTF.Exp, scale=scale
                            )
                        if ma % 2 == 1:
                            penda.append((ma // 2, exa))
                    if mb >= 0:
                        if mb % 2 == 0:
                            exb = epool.tile([128, 2, SQ], F8E4, name="exb", tag="ex")
                        for n in range(NQ):
                            nsl = slice(n * 512, (n + 1) * 512)
                            psb = spool.tile(
                                [128, 512], F32, name=f"psb{n}", tag=f"sb{n}", bufs=1
                            )
                            score_half(psb, 64, mb, n)
                            if mb in (3,):
                                # DVE (exp + recip + normalize) carries more
                                # than ACT; shift one B-exp over to balance
                                nc.scalar.activation(
                                    exb[:, mb % 2, nsl], psb[:, :], ACTF.Exp,
                                    scale=scale,
                                )
                            else:
                                nc.vector.tensor_scalar(
                                    exb.bitcast(I8)[:, mb % 2, nsl], psb[:, :],
                                    sch8_mul, sch8_add, ALU.mult, ALU.add,
                                )
                        if mb % 2 == 1:
                            pendb.append((mb // 2, exb))
                    if tail_prev is not None:
                        if step == 2:
                            tail_prev[0]()
                        elif step == 3:
                            tail_prev[1]()
                        elif step == 4:
                            tail_prev[2]()
                    # drain ctx (ktile-pair granularity) behind scores; with a
                    # handed-off tail, hold the backlog until the pc banks are
                    # freed by the tail's last reads (pcb: s1 evict @2;
                    # pca: s3 psum-direct multiply @4)
                    ok_a = tail_prev is None or step >= 5
                    ok_b = tail_prev is None or step >= 3
                    if ok_a:
                        while len(penda) > (1 if ma < KT8 else 0):
                            p_, ex_ = penda.pop(0)
                            emit_ctx_pair(pca, p_, ex_, 2 * t)
                    if ok_b:
                        while len(pendb) > (1 if mb < KT8 - 1 else 0):
                            p_, ex_ = pendb.pop(0)
                            emit_ctx_pair(pcb, p_, ex_, 2 * t + 1)

                # ---- this pair's tail (emitted by the NEXT pair) ----
                # Head B's ctx+den evicts to SBUF (ACT) because its rows must
                # cross partitions (SBUF->SBUF DMA shift).  Head A's ctx stays
                # in PSUM: the den reciprocals read PSUM directly on DVE (the
                # recip doubles as the evict), 1/den partition-broadcasts via
                # a DRAM bounce, and the normalize multiplies read pca from
                # PSUM (DVE) / cb from SBUF (GPSIMD).
                cb = pairpool.tile([128, SQ], F32, name="cb", tag="cb")
                rb = pairpool.tile([128, SQ], F32, name="rb", tag="rb")
                scra = rpool.tile([128, SQ], F32, name="scra", tag="scra")
                scrb = rpool.tile([128, SQ], F32, name="scrb", tag="scrb")
                cn = pairpool.tile([128, SQ], BF16, name="cn", tag="cn")

                def s1():
                    nc.scalar.activation(cb[0:DH + 1, :], pcb[0:DH + 1, :], ACTF.Copy)

                def s2():
                    # NOTE: the custom-DVE reciprocal op silently corrupts at
                    # a non-zero base partition, so run it over rows 0..64
                    # (same cost: DVE time scales with free size, not rows)
                    # and use only the den row 64 downstream.
                    if t == 3:
                        # Last pair: the whole normalize chain is exposed at
                        # the attend->FFN boundary, so skip the ~5us DRAM
                        # bounce: partition-broadcast 1/den with two tiny
                        # f32r ones-matmuls per half (stationary at PE row
                        # group 64, col groups 0/64), evict to rb, and run the
                        # per-half normalize on DVE immediately.
                        for n in range(NQ):
                            nsl = slice(n * 512, (n + 1) * 512)
                            nc.vector.reciprocal_approx_fast(
                                scra[0:DH + 1, nsl], pca[0:DH + 1, nsl]
                            )
                            nc.vector.reciprocal_approx_fast(
                                scrb[0:DH + 1, nsl], cb[0:DH + 1, nsl]
                            )
                        nc.gpsimd.dma_start(out=cb[64:128, :], in_=cb[0:64, :])
                        rbps = _work_pair("rbps")
                        for n in range(NQ):
                            nsl = slice(n * 512, (n + 1) * 512)
                            for cg, scr in ((0, scra), (64, scrb)):
                                nc.tensor.matmul(
                                    rbps[n][cg:cg + 64, :],
                                    ones64[DH:DH + 1, 0:64],
                                    scr[DH:DH + 1, nsl],
                                    start=True, stop=True,
                                )
                            nc.scalar.activation(
                                rb[:, nsl], rbps[n][:, :], ACTF.Copy
                            )
                            nc.vector.tensor_mul(
                                cn[0:64, nsl], pca[0:DH, nsl], rb[0:64, nsl]
                            )
                            nc.vector.tensor_mul(
                                cn[64:128, nsl], cb[64:128, nsl], rb[64:128, nsl]
                            )
                            nc.vector.tensor_add(
                                o1[:, t, nsl], cn[:, nsl], qpb[:, t, nsl]
                            )
                        return
                    nc.vector.reciprocal_approx_fast(
                        scra[0:DH + 1, :], pca[0:DH + 1, :]
                    )
                    nc.vector.reciprocal_approx_fast(
                        scrb[0:DH + 1, :], cb[0:DH + 1, :]
                    )
                    for hh, scr in ((0, scra), (1, scrb)):
                        rec_d = dpool.tile([1, SQ], F32, name="rec_d", tag="rec_d")
                        nc.sync.dma_start(out=rec_d[:, :], in_=scr[DH:DH + 1, :])
                        bsrc = bass.AP(
                            tensor=rec_d[0:1, :].tensor,
                            offset=rec_d[0:1, :].offset,
                            ap=[[0, 64], [1, SQ]],
                        )
                        nc.sync.dma_start(out=rb[64 * hh:64 * hh + 64, :], in_=bsrc)
                    nc.gpsimd.dma_start(out=cb[64:128, :], in_=cb[0:64, :])

                def s3():
                    if t == 3:
                        return  # folded into s2 for the last pair
                    nc.vector.tensor_mul(cn[0:64, :], pca[0:DH, :], rb[0:64, :])
                    nc.gpsimd.tensor_mul(
                        cn[64:128, :], cb[64:128, :], rb[64:128, :]
                    )
                    nc.gpsimd.tensor_add(o1[:, t, :], cn[:, :], qpb[:, t, :])
                    if dbg and t == 0:
                        nc.sync.dma_start(out=dDRB[:, :], in_=rb[:, :])
                        nc.sync.dma_start(out=dDCB[:, :], in_=cb[:, :])
                        nc.sync.dma_start(out=dDCN[:, :], in_=cn[:, :])

                return [s1, s2, s3]

            # ---- FFN: out = O1 + bv + relu(WoT.T @ O1 + bo2) ----
            def ffn_p1(m, pf=None):
                """kc0-2 partial passes into a pair of [128,512] psum halves."""
                if pf is None:
                    pf = _work_pair("pf")
                for kc in range(KC - 1):
                    for n in range(NQ):
                        nsl = slice(n * 512, (n + 1) * 512)
                        nc.tensor.matmul(
                            pf[n][:, :],
                            wo[:, kc, m * 128:(m + 1) * 128],
                            o1[:, kc, nsl],
                            start=(kc == 0),
                            stop=False,
                        )
                return pf

            def ffn_p2(m, pf):
                """kc3 pass + relu/residual/store, pipelined in 512-halves so
                the store of the first half overlaps the math of the second."""
                rf = outpool.tile([128, SQ], F32, name="rf", tag="rf")
                ot = outpool.tile([128, SQ], BF16, name="ot", tag="ot")
                for n in range(NQ):
                    nsl = slice(n * 512, (n + 1) * 512)
                    nc.tensor.matmul(
                        pf[n][:, :],
                        wo[:, KC - 1, m * 128:(m + 1) * 128],
                        o1[:, KC - 1, nsl],
                        start=False,
                        stop=True,
                    )
                    nc.scalar.activation(
                        rf[:, nsl], pf[n][:, :], ACTF.Relu, bias=bo2[:, m:m + 1]
                    )
                    nc.vector.scalar_tensor_tensor(
                        ot[:, nsl], rf[:, nsl], bv[:, m:m + 1], o1[:, m, nsl],
                        ALU.add, ALU.add,
                    )
                    eng = nc.gpsimd if (m + n) % 2 == 0 else nc.sync
                    eng.dma_start(
                        out=dOT[m * 128:(m + 1) * 128, nsl], in_=ot[:, nsl]
                    )

            # ---- emission ----
            # all of K-proj first (its inputs land first); Q/V after
            project(kpb, wk, kt, bk[:, 0:1], 0, dr=True)
            project(kpb, wk, kt, bk[:, 1:2], 1, dr=True)
            project(kpb, wk, kt, bk[:, 2:3], 2, dr=True)
            project(kpb, wk, kt, bk[:, 3:4], 3, dr=True)
            project(qpb, wq, qt, bq[:, 0:1], 0)
            project_v2(0)
            project(qpb, wq, qt, bq[:, 1:2], 1)
            project_v2(1)
            project(qpb, wq, qt, bq[:, 2:3], 2)
            project_v2(2)
            project(qpb, wq, qt, bq[:, 3:4], 3)
            project_v2(3)

            tail = attend_pair(0, None)
            tail = attend_pair(1, tail)
            tail = attend_pair(2, tail)
            tail = attend_pair(3, tail)
            # interleave the last pair's tail with ALL FFN kc0-2 passes: the
            # tail's recip/broadcast/normalize latency (several us) is hidden
            # behind 24 matmuls that only need o1[:, 0:3, :], and no ffn_p2
            # (which needs o1[:, 3, :]) sits in the PE queue ahead of them.
            tail[0]()
            pf0 = ffn_p1(0)
            tail[1]()
            # tail[1] allocated its rbps broadcast pair on the other ring
            # side; skip that side for pf1 (its banks free after the rb
            # evicts, which are already emitted) instead of colliding with
            # pf0's still-unread side
            _pp_flip[0] ^= 1
            pf1 = ffn_p1(1)
            tail[2]()
            # m=2/3 partials reuse the last attend pair's (now drained)
            # ctx-accumulator banks; m=0/1 hold the four score-ring half-banks
            _pfull2 = cpool.tile([128, SQ], F32, name="pf2", tag="pc")
            pf2 = ffn_p1(2, [_pfull2[:, 0:512], _pfull2[:, 512:1024]])
            _pfull3 = cpool.tile([128, SQ], F32, name="pf3", tag="pc")
            pf3 = ffn_p1(3, [_pfull3[:, 0:512], _pfull3[:, 512:1024]])

            if dbg:
                nc.sync.dma_start(out=dDQP[:, :, :], in_=qpb[:, :, :])
                nc.sync.dma_start(out=dDKP[:, :, :], in_=kpb[:, :, :])
                nc.sync.dma_start(out=dDVPA[:, :, :, :], in_=vpa[:, :, :, :])
                nc.sync.dma_start(out=dDO1[:, :, :], in_=o1[:, :, :])

            ffn_p2(0, pf0)
            ffn_p2(1, pf1)
            ffn_p2(2, pf2)
            ffn_p2(3, pf3)

    nc.compile()
    return nc


def _get_nc():
    global _NC
    if _NC is None:
        _NC = _build()
    return _NC


def _prep_inputs(Q, K, Wq, bq, Wk, bk, Wv, bv, Wo, bo):
    Q = np.asarray(Q, dtype=np.float32)
    K = np.asarray(K, dtype=np.float32)
    Wq = np.asarray(Wq, dtype=np.float32)
    Wk = np.asarray(Wk, dtype=np.float32)
    Wv = np.asarray(Wv, dtype=np.float32)
    Wo = np.asarray(Wo, dtype=np.float32)
    bq = np.asarray(bq, dtype=np.float32)
    bk = np.asarray(bk, dtype=np.float32)
    bv = np.asarray(bv, dtype=np.float32)
    bo = np.asarray(bo, dtype=np.float32)

    bo2 = (bo + Wo @ bv).astype(np.float32)

    def btile(b):
        return np.ascontiguousarray(b.reshape(MT, 128).T)

    import ml_dtypes
    bf = ml_dtypes.bfloat16
    f8 = mybir.dt.np(F8E4)
    shared = {
        "WqT": np.ascontiguousarray(Wq.T).astype(bf),
        "WkT": np.ascontiguousarray(Wk.T).astype(f8),
        "WvT": np.ascontiguousarray(Wv.T).astype(f8),
        "WoT": np.ascontiguousarray(Wo.T).astype(bf),
        "BIAS": np.ascontiguousarray(
            np.stack([btile(bk), btile(bq), btile(bo2), btile(bv)], axis=1)
        ),
    }
    in_maps = []
    for c in range(N_CORES):
        m = dict(shared)
        m["QT"] = np.ascontiguousarray(Q[c].T).astype(bf)
        m["KT"] = np.ascontiguousarray(K[c].T).astype(f8)
        in_maps.append(m)
    return in_maps


def run(inputs, trace=False):
    """Run on hardware; returns (output [B,SQ,D] f32, BassKernelResults)."""
    in_maps = _prep_inputs(
        inputs["Q"], inputs["K"], inputs["Wq"], inputs["bq"], inputs["Wk"],
        inputs["bk"], inputs["Wv"], inputs["bv"], inputs["Wo"], inputs["bo"],
    )
    nc = _get_nc()
    res = run_bass_kernel_spmd(
        nc, in_maps, core_ids=list(range(N_CORES)), trace=trace
    )
    out = np.stack(
        [res.results[c]["OT"].T for c in range(N_CORES)], axis=0
    ).astype(np.float32)
    return out, res


def kernel(**inputs):
    nh = inputs.get("num_heads", H)
    assert int(nh) == H, f"kernel hardcodes num_heads={H}, got {nh}"
    out, _ = run(inputs, trace=False)
    return out


if __name__ == "__main__":
    rng = np.random.default_rng(0)
    inputs = {
        "Q": rng.standard_normal((B, SQ, D), dtype=np.float32),
        "K": rng.standard_normal((B, SK, D), dtype=np.float32),
        "Wq": rng.standard_normal((D, D), dtype=np.float32) * 0.04,
        "bq": rng.standard_normal((D,), dtype=np.float32) * 0.04,
        "Wk": rng.standard_normal((D, D), dtype=np.float32) * 0.04,
        "bk": rng.standard_normal((D,), dtype=np.float32) * 0.04,
        "Wv": rng.standard_normal((D, D), dtype=np.float32) * 0.04,
        "bv": rng.standard_normal((D,), dtype=np.float32) * 0.04,
        "Wo": rng.standard_normal((D, D), dtype=np.float32) * 0.04,
        "bo": rng.standard_normal((D,), dtype=np.float32) * 0.04,
        "num_heads": H,
    }
    out = kernel(**inputs)
    print("out", out.shape, out.dtype, float(np.abs(out).max()))

